# revision 35
# baseline (speedup 1.0000x reference)
"""Causal multi-head attention block (qkv proj + attention + out proj) on 8 TRN2 cores.

Problem: x[4,2048,1024] @ Wqkv[1024,3072] -> 16-head causal attention -> @ Wout.

Sharding: batch(4) x head-group(2) -> 8 cores. Core c handles batch c//2 and
heads (c%2)*8..(c%2)*8+8. Each core computes its 8 heads' attention and a
partial out-projection [2048,1024]; host sums the two head-group partials per
batch and adds bout.

Device kernel v2 (per core):
  Projections in f32r (K=128 stationary tiles stream at ~216ns/512 rows).
  Q is stored zero-padded per head (other head's 64 rows = 0) so the S matmul
  keeps a K=128 stationary tile (K=64 stationaries reload ~2x slower).
  S^T[k,q] tiles f32r with causal q-trimming; exp on ACT -> bf16 SBUF ring;
  diagonal 128-col chunk masked via one shared triangular bf16 mask on DVE.
  ctx accumulated in [q, 65] layout: bf16 N=65 matmuls (expS chunk as lhsT,
  [V|1] as rhs); col 64 = softmax denominator; normalize via strided
  reciprocal + per-partition scalar multiply -> cn[q, feat] bf16.
  cn transposed on PE (bf16, via identity) -> cnT[feat, q]; out-projection
  bf16 with wout chunks as moving operand -> Y[tok, out] f32.
"""
import numpy as np

B, T, C = 4, 2048, 1024
H, HD = 16, 64
NCORES = 8


def _build_program():
    import concourse.bacc as bacc
    import concourse.tile as tile
    from concourse import mybir, masks as cmasks

    dtf = mybir.dt.float32
    dtr = mybir.dt.float32r
    dtb = mybir.dt.bfloat16
    EXP = mybir.ActivationFunctionType.Exp
    MULT = mybir.AluOpType.mult

    nc = bacc.Bacc('TRN2', target_bir_lowering=False, debug=False)
    xt_d = nc.dram_tensor("xt", [1024, 2048], dtr, kind="ExternalInput").ap()
    wqk_d = nc.dram_tensor("wqk", [1024, 1024], dtr, kind="ExternalInput").ap()
    wv_d = nc.dram_tensor("wv", [1024, 512], dtr, kind="ExternalInput").ap()
    wout_d = nc.dram_tensor("wout", [512, 1024], dtb, kind="ExternalInput").ap()
    mask_d = nc.dram_tensor("mask", [128, 128], dtb, kind="ExternalInput").ap()
    bqk_d = nc.dram_tensor("bqk", [1024], dtf, kind="ExternalInput").ap()
    bv_d = nc.dram_tensor("bv", [512], dtf, kind="ExternalInput").ap()
    y_d = nc.dram_tensor("y", [2048, 1024], dtf, kind="ExternalOutput").ap()

    with tile.TileContext(nc) as tc:
        with tc.tile_pool(name="const", bufs=1) as const, \
             tc.tile_pool(name="xt_p", bufs=8) as xt_p, \
             tc.tile_pool(name="ring", bufs=12) as ring_p, \
             tc.tile_pool(name="mx_p", bufs=4) as mx_p, \
             tc.tile_pool(name="cn_p", bufs=8) as cn_p, \
             tc.tile_pool(name="cnT_p", bufs=8) as cnT_p, \
             tc.tile_pool(name="r_p", bufs=3) as r_p, \
             tc.tile_pool(name="y_p", bufs=3) as y_p, \
             tc.tile_pool(name="ps_s", bufs=2, space="PSUM") as ps_s, \
             tc.tile_pool(name="ps_ctx", bufs=2, space="PSUM") as ps_ctx, \
             tc.tile_pool(name="ps_big", bufs=2, space="PSUM") as ps_big:

            # ---- constants / weights ----
            wqk_sb = const.tile([128, 8, 8, 128], dtr)   # [p, kc, j, c]
            wv_sb = const.tile([128, 8, 512], dtr)       # [p, kc, n]
            wout_sb = const.tile([128, 4, 2, 512], dtb)  # [p, fc, oc, c]
            maskb = const.tile([128, 128], dtb)
            bqk_sb = const.tile([128, 8], dtf)
            bv_sb = const.tile([128, 4], dtf)
            ident = const.tile([128, 128], dtb)
            vcol_f = const.tile([128, 16, 8, 1], dtf)
            kt_store = const.tile([128, 4, 4, 512], dtr)  # [p, j, tt, t]
            v_all = const.tile([128, 16, 8, 65], dtb)     # [p, kt, h, d|1]
            qs0 = const.tile([128, 4, 2, 512], dtr)       # [p, hp, hb, q]
            qs1 = const.tile([128, 4, 2, 512], dtr)
            qs = [qs0, qs1]

            # interleave x-tile-0 and j-major weight loads in consumption
            # order: chain (j=0, kc) needs xt[kc] + wqk j0 only
            xts0 = []
            xt_t0 = xt_p.tile([128, 512], dtr, tag="xt", name="xt_t0")
            nc.sync.dma_start(xt_t0[:], xt_d[0:128, 0:512])
            xts0.append(xt_t0)
            nc.sync.dma_start(wqk_sb[:, :, 0, :],
                              wqk_d[:, 0:128]
                              .rearrange("(kc p) c -> p kc c", p=128))
            for kc in range(1, 8):
                xt_t0 = xt_p.tile([128, 512], dtr, tag="xt", name="xt_t0")
                nc.sync.dma_start(xt_t0[:], xt_d[kc * 128:(kc + 1) * 128, 0:512])
                xts0.append(xt_t0)
            nc.sync.dma_start(bqk_sb[:], bqk_d.rearrange("(j p) -> p j", p=128))
            for j in range(1, 8):
                nc.sync.dma_start(wqk_sb[:, :, j, :],
                                  wqk_d[:, j * 128:(j + 1) * 128]
                                  .rearrange("(kc p) c -> p kc c", p=128))
            nc.sync.dma_start(wv_sb[:],
                              wv_d.rearrange("(kc p) n -> p kc n", p=128))
            nc.sync.dma_start(wout_sb[:],
                              wout_d.rearrange("(fc p) (oc c) -> p fc oc c",
                                               p=128, c=512))
            nc.sync.dma_start(maskb[:], mask_d)
            nc.sync.dma_start(bv_sb[:], bv_d.rearrange("(fc p) -> p fc", p=128))
            cmasks.make_identity(nc, ident[:])
            nc.vector.memset(vcol_f[:], 1.0)
            nc.vector.tensor_copy(v_all[:, :, :, 64:65], vcol_f[:])
            zt = const.tile([128, 512], dtf)
            nc.vector.memset(zt[:], 0.0)
            zb = const.tile([128, 128], dtb)
            nc.vector.memset(zb[:], 0.0)
            junk260 = const.tile([128, 260], dtb)
            nc.vector.memset(junk260[:], 1.0)
            for q_ in (qs0, qs1):
                for j_ in range(4):
                    for hb_ in range(2):
                        nc.vector.tensor_copy(q_[:, j_, hb_, :], zt[:])

            def dma_xt(tt, xts):
                for kc in range(8):
                    xt_t = xt_p.tile([128, 512], dtr, tag="xt", name="xt_t")
                    nc.sync.dma_start(
                        xt_t[:], xt_d[kc * 128:(kc + 1) * 128,
                                      tt * 512:(tt + 1) * 512])
                    xts.append(xt_t)

            def proj_q(tt, xts):
                par = tt % 2
                for j in range(4):
                    psq = ps_big.tile([128, 512], dtf, tag="big", name="psq")
                    for kc in range(8):
                        nc.tensor.matmul(psq[:], wqk_sb[:, kc, j, :],
                                         xts[kc][:], start=(kc == 0),
                                         stop=(kc == 7)).annotate('mm_q')
                    nc.vector.tensor_scalar_add(
                        qs[par][0:64, j, 0, :], psq[0:64, :],
                        bqk_sb[0:64, j:j + 1])
                    nc.vector.tensor_scalar_add(
                        qs[par][64:128, j, 1, :], psq[64:128, :],
                        bqk_sb[64:128, j:j + 1])

            def proj_kv(tt, xts):
                for j in range(4):
                    psk = ps_big.tile([128, 512], dtf, tag="big", name="psk")
                    for kc in range(8):
                        nc.tensor.matmul(psk[:], wqk_sb[:, kc, 4 + j, :],
                                         xts[kc][:], start=(kc == 0),
                                         stop=(kc == 7)).annotate('mm_k')
                    nc.vector.tensor_scalar_add(kt_store[:, j, tt, :], psk[:],
                                                bqk_sb[:, 4 + j:5 + j])
                for sub in range(4):
                    vt = tt * 4 + sub
                    psv = ps_big.tile([128, 512], dtf, tag="big", name="psv")
                    for kc in range(8):
                        nc.tensor.matmul(psv[:],
                                         xts[kc][:, sub * 128:(sub + 1) * 128],
                                         wv_sb[:, kc, :],
                                         start=(kc == 0),
                                         stop=(kc == 7)).annotate('mm_v')
                    nc.vector.tensor_copy(
                        v_all[:, vt, :, 0:64],
                        psv[:].rearrange("p (h d) -> p h d", h=8))

            def attention_hp(qt, hp, cn_tiles):
                par = qt % 2
                n_kt = 4 * qt + 4
                ctx_of = {}
                for hb in range(2):
                    cx = ps_ctx.tile([128, 260], dtf, tag="ctx", name="cx")
                    # zero the bank with one matmul, then accumulate with
                    # start=False only; interleaved per-region start/stop
                    # chains lose earlier chains' in-flight rounds (see mb8).
                    nc.tensor.matmul(cx[:, 0:260], zb[:], junk260[:],
                                     start=True, stop=False,
                                     skip_group_check=True).annotate('mm_z')
                    ctx_of[hb] = cx
                for ktg in range(n_kt):
                    ktt, kj = ktg // 4, ktg % 4
                    di = ktg - 4 * qt
                    if di < 0:
                        off = 0
                    else:
                        off = min(128 * di, 256)
                    nn = 512 - off
                    lhsT = kt_store[:, hp, ktt, kj * 128:(kj + 1) * 128]
                    # both heads' S in one 2-bank psum pair; one (or two)
                    # ACT exp instruction(s) covering both
                    s_ps = ps_s.tile([128, 1024], dtf, tag="s", name="s_ps")
                    e = ring_p.tile([128, 1024], dtb, tag="e", name="e")
                    for hb in range(2):
                        nc.tensor.matmul(
                            s_ps[:, 512 * hb:512 * hb + nn], lhsT,
                            qs[par][:, hp, hb, off:512],
                            start=True, stop=True).annotate('mm_s')
                    if di < 0:
                        rc_d, qc0 = 0, 0
                    else:
                        qc0 = di
                        rc_d = 128 * di - off
                    ne = nn - rc_d
                    if di < 2:
                        # single exp across both banks (incl. dead gap)
                        nc.scalar.activation(e[:, rc_d:512 + rc_d + ne],
                                             s_ps[:, rc_d:512 + rc_d + ne],
                                             EXP)
                    else:
                        nc.scalar.activation(e[:, rc_d:rc_d + ne],
                                             s_ps[:, rc_d:rc_d + ne], EXP)
                        nc.scalar.activation(e[:, 512 + rc_d:512 + rc_d + ne],
                                             s_ps[:, 512 + rc_d:512 + rc_d + ne],
                                             EXP)
                    mexps = {}
                    if di >= 0:
                        for hb in range(2):
                            mexp = mx_p.tile([128, 128], dtb, tag="mx",
                                             name="mexp")
                            nc.vector.tensor_tensor(
                                mexp[:], e[:, 512 * hb + rc_d:512 * hb + rc_d + 128],
                                maskb[:], MULT)
                            mexps[hb] = mexp
                    for hb in range(2):
                        head = 2 * hp + hb
                        for qc in range(qc0, 4):
                            rc = 128 * qc - off
                            lh = (mexps[hb][:] if (di >= 0 and qc == qc0)
                                  else e[:, 512 * hb + rc:512 * hb + rc + 128])
                            nc.tensor.matmul(
                                ctx_of[hb][:, qc * 65:qc * 65 + 65],
                                lh, v_all[:, ktg, head, :],
                                start=False, stop=False,
                                skip_group_check=True).annotate('mm_ctx')
                # normalize
                for hb in range(2):
                    head = 2 * hp + hb
                    rcp = r_p.tile([128, 4], dtf, tag="r", name="rcp")
                    nc.vector.reciprocal_approx_fast(
                        rcp[:], ctx_of[hb][:, 64:260:65])
                    for qc in range(4):
                        nc.vector.tensor_scalar_mul(
                            cn_tiles[qc][:, head * 64:head * 64 + 64],
                            ctx_of[hb][:, qc * 65:qc * 65 + 64],
                            rcp[:, qc:qc + 1])

            def transp_out_half(qt, cn_tiles, cnT_tiles, mis):
                if not cnT_tiles:
                    for fc in range(4):
                        tp = ps_big.tile([128, 512], dtb, tag="big", name="tp",
                                         padded_shape=[128, 512])
                        for qc in range(4):
                            nc.tensor.transpose(
                                tp[:, qc * 128:(qc + 1) * 128],
                                cn_tiles[qc][:, fc * 128:(fc + 1) * 128],
                                ident[:]).annotate('mm_t')
                        cnT = cnT_p.tile([128, 512], dtb, tag="ct", name="cnT")
                        nc.vector.tensor_scalar_add(cnT[:], tp[:],
                                                    bv_sb[:, fc:fc + 1])
                        cnT_tiles.append(cnT)
                for mi in mis:
                    for oc in range(2):
                        psy = ps_big.tile([128, 512], dtf, tag="big",
                                          name="psy", padded_shape=[128, 512])
                        for fc in range(4):
                            nc.tensor.matmul(
                                psy[:],
                                cnT_tiles[fc][:, mi * 128:(mi + 1) * 128],
                                wout_sb[:, fc, oc, :],
                                start=(fc == 0), stop=(fc == 3)).annotate('mm_y')
                        y_sb = y_p.tile([128, 512], dtf, tag="y", name="y_sb")
                        nc.vector.tensor_copy(y_sb[:], psy[:])
                        nc.sync.dma_start(
                            y_d[qt * 512 + mi * 128: qt * 512 + (mi + 1) * 128,
                                oc * 512:(oc + 1) * 512],
                            y_sb[:])

            # ---- main schedule ----
            proj_q(0, xts0)
            proj_kv(0, xts0)
            cns_prev = None
            cnT_prev = None
            for qt in range(4):
                cn_tiles = [cn_p.tile([128, 512], dtb, tag="cn", name="cn")
                            for _ in range(4)]
                xts_next = []
                if qt < 3:
                    dma_xt(qt + 1, xts_next)
                for hp in range(4):
                    attention_hp(qt, hp, cn_tiles)
                    if hp == 0 and qt < 3:
                        proj_q(qt + 1, xts_next)
                    elif hp == 1 and qt < 3:
                        proj_kv(qt + 1, xts_next)
                    if cns_prev is not None:
                        if hp == 0:
                            cnT_prev = []
                        transp_out_half(qt - 1, cns_prev, cnT_prev, (hp,))
                cns_prev = cn_tiles
            cnT_last = []
            transp_out_half(3, cns_prev, cnT_last, (0, 1, 2, 3))
    nc.compile()
    return nc


def _host_shards(x, Wqkv, bqkv, Wout):
    import ml_dtypes
    kk = np.arange(128)[:, None]
    qq = np.arange(128)[None, :]
    mask = (qq >= kk).astype(ml_dtypes.bfloat16)

    in_maps = []
    for c in range(NCORES):
        b, hg = c // 2, c % 2
        s = hg * 512
        xt = np.ascontiguousarray(x[b].T)
        wqk = np.ascontiguousarray(
            np.concatenate([Wqkv[:, s:s + 512] * 0.125,
                            Wqkv[:, 1024 + s:1024 + s + 512]], axis=1))
        wv = np.ascontiguousarray(Wqkv[:, 2048 + s:2048 + s + 512])
        wout = np.ascontiguousarray(Wout[s:s + 512, :]).astype(ml_dtypes.bfloat16)
        bqk = np.concatenate([bqkv[s:s + 512] * 0.125,
                              bqkv[1024 + s:1024 + s + 512]]).astype(np.float32)
        bv = np.ascontiguousarray(bqkv[2048 + s:2048 + s + 512]).astype(np.float32)
        in_maps.append({"xt": xt, "wqk": wqk, "wv": wv, "wout": wout,
                       "mask": mask, "bqk": bqk, "bv": bv})
    return in_maps


_CACHED = {}


def kernel(x, Wqkv, bqkv, Wout, bout):
    from concourse.bass_utils import run_bass_kernel_spmd

    x = np.asarray(x, dtype=np.float32)
    Wqkv = np.asarray(Wqkv, dtype=np.float32)
    bqkv = np.asarray(bqkv, dtype=np.float32)
    Wout = np.asarray(Wout, dtype=np.float32)
    bout = np.asarray(bout, dtype=np.float32)
    assert x.shape == (B, T, C), x.shape

    if 'nc' not in _CACHED:
        _CACHED['nc'] = _build_program()
    nc = _CACHED['nc']

    in_maps = _host_shards(x, Wqkv, bqkv, Wout)
    res = run_bass_kernel_spmd(nc, in_maps, core_ids=list(range(NCORES)))

    y = np.empty((B, T, C), np.float32)
    for b in range(B):
        y[b] = res.results[2 * b]["y"] + res.results[2 * b + 1]["y"] + bout
    return y


# revision 36
# speedup vs baseline: 1.0245x; 1.0245x over previous
"""Causal multi-head attention block (qkv proj + attention + out proj) on 8 TRN2 cores.

Problem: x[4,2048,1024] @ Wqkv[1024,3072] -> 16-head causal attention -> @ Wout.

Sharding: batch(4) x head-group(2) -> 8 cores. Core c handles batch c//2 and
heads (c%2)*8..(c%2)*8+8. Each core computes its 8 heads' attention and a
partial out-projection [2048,1024]; host sums the two head-group partials per
batch and adds bout.

Device kernel v2 (per core):
  Projections in f32r (K=128 stationary tiles stream at ~216ns/512 rows).
  Q is stored zero-padded per head (other head's 64 rows = 0) so the S matmul
  keeps a K=128 stationary tile (K=64 stationaries reload ~2x slower).
  S^T[k,q] tiles f32r with causal q-trimming; exp on ACT -> bf16 SBUF ring;
  diagonal 128-col chunk masked via one shared triangular bf16 mask on DVE.
  ctx accumulated in [q, 65] layout: bf16 N=65 matmuls (expS chunk as lhsT,
  [V|1] as rhs); col 64 = softmax denominator; normalize via strided
  reciprocal + per-partition scalar multiply -> cn[q, feat] bf16.
  cn transposed on PE (bf16, via identity) -> cnT[feat, q]; out-projection
  bf16 with wout chunks as moving operand -> Y[tok, out] f32.
"""
import numpy as np

B, T, C = 4, 2048, 1024
H, HD = 16, 64
NCORES = 8


def _build_program():
    import concourse.bacc as bacc
    import concourse.tile as tile
    from concourse import mybir, masks as cmasks

    dtf = mybir.dt.float32
    dtr = mybir.dt.float32r
    dtb = mybir.dt.bfloat16
    EXP = mybir.ActivationFunctionType.Exp
    MULT = mybir.AluOpType.mult

    nc = bacc.Bacc('TRN2', target_bir_lowering=False, debug=False)
    xt_d = nc.dram_tensor("xt", [1024, 2048], dtr, kind="ExternalInput").ap()
    wqk_d = nc.dram_tensor("wqk", [1024, 1024], dtr, kind="ExternalInput").ap()
    wv_d = nc.dram_tensor("wv", [1024, 512], dtr, kind="ExternalInput").ap()
    wout_d = nc.dram_tensor("wout", [512, 1024], dtb, kind="ExternalInput").ap()
    mask_d = nc.dram_tensor("mask", [128, 128], dtb, kind="ExternalInput").ap()
    bqk_d = nc.dram_tensor("bqk", [1024], dtf, kind="ExternalInput").ap()
    bv_d = nc.dram_tensor("bv", [512], dtf, kind="ExternalInput").ap()
    y_d = nc.dram_tensor("y", [2048, 1024], dtf, kind="ExternalOutput").ap()

    with tile.TileContext(nc) as tc:
        with tc.tile_pool(name="const", bufs=1) as const, \
             tc.tile_pool(name="xt_p", bufs=8) as xt_p, \
             tc.tile_pool(name="ring", bufs=12) as ring_p, \
             tc.tile_pool(name="mx_p", bufs=4) as mx_p, \
             tc.tile_pool(name="cn_p", bufs=8) as cn_p, \
             tc.tile_pool(name="cnT_p", bufs=8) as cnT_p, \
             tc.tile_pool(name="r_p", bufs=3) as r_p, \
             tc.tile_pool(name="y_p", bufs=3) as y_p, \
             tc.tile_pool(name="ps_s", bufs=2, space="PSUM") as ps_s, \
             tc.tile_pool(name="ps_ctx", bufs=2, space="PSUM") as ps_ctx, \
             tc.tile_pool(name="ps_big", bufs=2, space="PSUM") as ps_big:

            # ---- constants / weights ----
            wqk_sb = const.tile([128, 8, 8, 128], dtr)   # [p, kc, j, c]
            wv_sb = const.tile([128, 8, 512], dtr)       # [p, kc, n]
            wout_sb = const.tile([128, 4, 2, 512], dtb)  # [p, fc, oc, c]
            maskb = const.tile([128, 128], dtb)
            bqk_sb = const.tile([128, 8], dtf)
            bv_sb = const.tile([128, 4], dtf)
            ident = const.tile([128, 128], dtb)
            vcol_f = const.tile([128, 16, 8, 1], dtf)
            kt_store = const.tile([128, 4, 4, 512], dtr)  # [p, j, tt, t]
            v_all = const.tile([128, 16, 8, 65], dtb)     # [p, kt, h, d|1]
            qs0 = const.tile([128, 4, 2, 512], dtr)       # [p, hp, hb, q]
            qs1 = const.tile([128, 4, 2, 512], dtr)
            qs = [qs0, qs1]

            # interleave x-tile-0 and j-major weight loads in consumption
            # order: chain (j=0, kc) needs xt[kc] + wqk j0 only
            xts0 = []
            xt_t0 = xt_p.tile([128, 512], dtr, tag="xt", name="xt_t0")
            nc.sync.dma_start(xt_t0[:], xt_d[0:128, 0:512])
            xts0.append(xt_t0)
            nc.sync.dma_start(wqk_sb[:, :, 0, :],
                              wqk_d[:, 0:128]
                              .rearrange("(kc p) c -> p kc c", p=128))
            for kc in range(1, 8):
                xt_t0 = xt_p.tile([128, 512], dtr, tag="xt", name="xt_t0")
                nc.sync.dma_start(xt_t0[:], xt_d[kc * 128:(kc + 1) * 128, 0:512])
                xts0.append(xt_t0)
            nc.sync.dma_start(bqk_sb[:], bqk_d.rearrange("(j p) -> p j", p=128))
            for j in range(1, 8):
                nc.sync.dma_start(wqk_sb[:, :, j, :],
                                  wqk_d[:, j * 128:(j + 1) * 128]
                                  .rearrange("(kc p) c -> p kc c", p=128))
            nc.sync.dma_start(wv_sb[:],
                              wv_d.rearrange("(kc p) n -> p kc n", p=128))
            nc.sync.dma_start(wout_sb[:],
                              wout_d.rearrange("(fc p) (oc c) -> p fc oc c",
                                               p=128, c=512))
            nc.sync.dma_start(maskb[:], mask_d)
            nc.sync.dma_start(bv_sb[:], bv_d.rearrange("(fc p) -> p fc", p=128))
            cmasks.make_identity(nc, ident[:])
            nc.vector.memset(vcol_f[:], 1.0)
            nc.vector.tensor_copy(v_all[:, :, :, 64:65], vcol_f[:])
            zt = const.tile([128, 512], dtf)
            nc.vector.memset(zt[:], 0.0)
            zb = const.tile([128, 128], dtb)
            nc.vector.memset(zb[:], 0.0)
            junk260 = const.tile([128, 260], dtb)
            nc.vector.memset(junk260[:], 1.0)
            for q_ in (qs0, qs1):
                for j_ in range(4):
                    for hb_ in range(2):
                        nc.vector.tensor_copy(q_[:, j_, hb_, :], zt[:])

            def dma_xt(tt, xts):
                for kc in range(8):
                    xt_t = xt_p.tile([128, 512], dtr, tag="xt", name="xt_t")
                    nc.sync.dma_start(
                        xt_t[:], xt_d[kc * 128:(kc + 1) * 128,
                                      tt * 512:(tt + 1) * 512])
                    xts.append(xt_t)

            def proj_q(tt, xts):
                par = tt % 2
                for j in range(4):
                    psq = ps_big.tile([128, 512], dtf, tag="big", name="psq")
                    for kc in range(8):
                        nc.tensor.matmul(psq[:], wqk_sb[:, kc, j, :],
                                         xts[kc][:], start=(kc == 0),
                                         stop=(kc == 7)).annotate('mm_q')
                    nc.vector.tensor_scalar_add(
                        qs[par][0:64, j, 0, :], psq[0:64, :],
                        bqk_sb[0:64, j:j + 1])
                    nc.vector.tensor_scalar_add(
                        qs[par][64:128, j, 1, :], psq[64:128, :],
                        bqk_sb[64:128, j:j + 1])

            def proj_kv(tt, xts):
                for j in range(4):
                    psk = ps_big.tile([128, 512], dtf, tag="big", name="psk")
                    for kc in range(8):
                        nc.tensor.matmul(psk[:], wqk_sb[:, kc, 4 + j, :],
                                         xts[kc][:], start=(kc == 0),
                                         stop=(kc == 7)).annotate('mm_k')
                    nc.vector.tensor_scalar_add(kt_store[:, j, tt, :], psk[:],
                                                bqk_sb[:, 4 + j:5 + j])
                for sub in range(4):
                    vt = tt * 4 + sub
                    psv = ps_big.tile([128, 512], dtf, tag="big", name="psv")
                    for kc in range(8):
                        nc.tensor.matmul(psv[:],
                                         xts[kc][:, sub * 128:(sub + 1) * 128],
                                         wv_sb[:, kc, :],
                                         start=(kc == 0),
                                         stop=(kc == 7)).annotate('mm_v')
                    nc.vector.tensor_copy(
                        v_all[:, vt, :, 0:64],
                        psv[:].rearrange("p (h d) -> p h d", h=8))

            def attention_hp(qt, hp, cn_tiles):
                par = qt % 2
                n_kt = 4 * qt + 4
                ctx_of = {}
                for hb in range(2):
                    cx = ps_ctx.tile([128, 260], dtf, tag="ctx", name="cx")
                    # zero the bank with one matmul, then accumulate with
                    # start=False only; interleaved per-region start/stop
                    # chains lose earlier chains' in-flight rounds (see mb8).
                    nc.tensor.matmul(cx[:, 0:260], zb[:], junk260[:],
                                     start=True, stop=False,
                                     skip_group_check=True).annotate('mm_z')
                    ctx_of[hb] = cx
                for ktg in range(n_kt):
                    ktt, kj = ktg // 4, ktg % 4
                    di = ktg - 4 * qt
                    if di < 0:
                        off = 0
                    else:
                        off = min(128 * di, 256)
                    nn = 512 - off
                    lhsT = kt_store[:, hp, ktt, kj * 128:(kj + 1) * 128]
                    # both heads' S in one 2-bank psum pair; one (or two)
                    # ACT exp instruction(s) covering both
                    s_ps = ps_s.tile([128, 1024], dtf, tag="s", name="s_ps")
                    e = ring_p.tile([128, 1024], dtb, tag="e", name="e")
                    for hb in range(2):
                        nc.tensor.matmul(
                            s_ps[:, 512 * hb:512 * hb + nn], lhsT,
                            qs[par][:, hp, hb, off:512],
                            start=True, stop=True).annotate('mm_s')
                    if di < 0:
                        rc_d, qc0 = 0, 0
                    else:
                        qc0 = di
                        rc_d = 128 * di - off
                    ne = nn - rc_d
                    if di < 2:
                        # single exp across both banks (incl. dead gap)
                        nc.scalar.activation(e[:, rc_d:512 + rc_d + ne],
                                             s_ps[:, rc_d:512 + rc_d + ne],
                                             EXP)
                    else:
                        nc.scalar.activation(e[:, rc_d:rc_d + ne],
                                             s_ps[:, rc_d:rc_d + ne], EXP)
                        nc.scalar.activation(e[:, 512 + rc_d:512 + rc_d + ne],
                                             s_ps[:, 512 + rc_d:512 + rc_d + ne],
                                             EXP)
                    mexps = {}
                    if di >= 0:
                        for hb in range(2):
                            mexp = mx_p.tile([128, 128], dtb, tag="mx",
                                             name="mexp")
                            nc.vector.tensor_tensor(
                                mexp[:], e[:, 512 * hb + rc_d:512 * hb + rc_d + 128],
                                maskb[:], MULT)
                            mexps[hb] = mexp
                    for hb in range(2):
                        head = 2 * hp + hb
                        for qc in range(qc0, 4):
                            rc = 128 * qc - off
                            lh = (mexps[hb][:] if (di >= 0 and qc == qc0)
                                  else e[:, 512 * hb + rc:512 * hb + rc + 128])
                            nc.tensor.matmul(
                                ctx_of[hb][:, qc * 65:qc * 65 + 65],
                                lh, v_all[:, ktg, head, :],
                                start=False, stop=False,
                                skip_group_check=True).annotate('mm_ctx')
                # normalize
                for hb in range(2):
                    head = 2 * hp + hb
                    rcp = r_p.tile([128, 4], dtf, tag="r", name="rcp")
                    nc.vector.reciprocal_approx_fast(
                        rcp[:], ctx_of[hb][:, 64:260:65])
                    for qc in range(4):
                        nc.vector.tensor_scalar_mul(
                            cn_tiles[qc][:, head * 64:head * 64 + 64],
                            ctx_of[hb][:, qc * 65:qc * 65 + 64],
                            rcp[:, qc:qc + 1])

            def transp_out_half(qt, cn_tiles, cnT_tiles, mis):
                if not cnT_tiles:
                    for fc in range(4):
                        tp = ps_big.tile([128, 512], dtb, tag="big", name="tp",
                                         padded_shape=[128, 512])
                        for qc in range(4):
                            nc.tensor.transpose(
                                tp[:, qc * 128:(qc + 1) * 128],
                                cn_tiles[qc][:, fc * 128:(fc + 1) * 128],
                                ident[:]).annotate('mm_t')
                        cnT = cnT_p.tile([128, 512], dtb, tag="ct", name="cnT")
                        nc.vector.tensor_scalar_add(cnT[:], tp[:],
                                                    bv_sb[:, fc:fc + 1])
                        cnT_tiles.append(cnT)
                for mi in mis:
                    for oc in range(2):
                        psy = ps_big.tile([128, 512], dtf, tag="big",
                                          name="psy", padded_shape=[128, 512])
                        for fc in range(4):
                            nc.tensor.matmul(
                                psy[:],
                                cnT_tiles[fc][:, mi * 128:(mi + 1) * 128],
                                wout_sb[:, fc, oc, :],
                                start=(fc == 0), stop=(fc == 3)).annotate('mm_y')
                        y_sb = y_p.tile([128, 512], dtf, tag="y", name="y_sb")
                        nc.vector.tensor_copy(y_sb[:], psy[:])
                        nc.sync.dma_start(
                            y_d[qt * 512 + mi * 128: qt * 512 + (mi + 1) * 128,
                                oc * 512:(oc + 1) * 512],
                            y_sb[:])

            # ---- main schedule ----
            proj_q(0, xts0)
            proj_kv(0, xts0)
            cns_prev = None
            cnT_prev = None
            for qt in range(4):
                cn_tiles = [cn_p.tile([128, 512], dtb, tag="cn", name="cn")
                            for _ in range(4)]
                xts_next = []
                if qt < 3:
                    dma_xt(qt + 1, xts_next)
                for hp in range(4):
                    attention_hp(qt, hp, cn_tiles)
                    if hp == 0 and qt < 3:
                        proj_q(qt + 1, xts_next)
                    elif hp == 1 and qt < 3:
                        proj_kv(qt + 1, xts_next)
                    if cns_prev is not None:
                        if qt == 3:
                            # no proj filler in qt3: spread outproj across hps
                            if hp == 0:
                                cnT_prev = []
                            transp_out_half(qt - 1, cns_prev, cnT_prev, (hp,))
                        elif hp == 2:
                            cnT_prev = []
                            transp_out_half(qt - 1, cns_prev, cnT_prev, (0, 1))
                        elif hp == 3:
                            transp_out_half(qt - 1, cns_prev, cnT_prev, (2, 3))
                cns_prev = cn_tiles
            cnT_last = []
            transp_out_half(3, cns_prev, cnT_last, (0, 1, 2, 3))
    nc.compile()
    return nc


def _host_shards(x, Wqkv, bqkv, Wout):
    import ml_dtypes
    kk = np.arange(128)[:, None]
    qq = np.arange(128)[None, :]
    mask = (qq >= kk).astype(ml_dtypes.bfloat16)

    in_maps = []
    for c in range(NCORES):
        b, hg = c // 2, c % 2
        s = hg * 512
        xt = np.ascontiguousarray(x[b].T)
        wqk = np.ascontiguousarray(
            np.concatenate([Wqkv[:, s:s + 512] * 0.125,
                            Wqkv[:, 1024 + s:1024 + s + 512]], axis=1))
        wv = np.ascontiguousarray(Wqkv[:, 2048 + s:2048 + s + 512])
        wout = np.ascontiguousarray(Wout[s:s + 512, :]).astype(ml_dtypes.bfloat16)
        bqk = np.concatenate([bqkv[s:s + 512] * 0.125,
                              bqkv[1024 + s:1024 + s + 512]]).astype(np.float32)
        bv = np.ascontiguousarray(bqkv[2048 + s:2048 + s + 512]).astype(np.float32)
        in_maps.append({"xt": xt, "wqk": wqk, "wv": wv, "wout": wout,
                       "mask": mask, "bqk": bqk, "bv": bv})
    return in_maps


_CACHED = {}


def kernel(x, Wqkv, bqkv, Wout, bout):
    from concourse.bass_utils import run_bass_kernel_spmd

    x = np.asarray(x, dtype=np.float32)
    Wqkv = np.asarray(Wqkv, dtype=np.float32)
    bqkv = np.asarray(bqkv, dtype=np.float32)
    Wout = np.asarray(Wout, dtype=np.float32)
    bout = np.asarray(bout, dtype=np.float32)
    assert x.shape == (B, T, C), x.shape

    if 'nc' not in _CACHED:
        _CACHED['nc'] = _build_program()
    nc = _CACHED['nc']

    in_maps = _host_shards(x, Wqkv, bqkv, Wout)
    res = run_bass_kernel_spmd(nc, in_maps, core_ids=list(range(NCORES)))

    y = np.empty((B, T, C), np.float32)
    for b in range(B):
        y[b] = res.results[2 * b]["y"] + res.results[2 * b + 1]["y"] + bout
    return y


# revision 37
# speedup vs baseline: 1.0315x; 1.0068x over previous
"""Causal multi-head attention block (qkv proj + attention + out proj) on 8 TRN2 cores.

Problem: x[4,2048,1024] @ Wqkv[1024,3072] -> 16-head causal attention -> @ Wout.

Sharding: batch(4) x head-group(2) -> 8 cores. Core c handles batch c//2 and
heads (c%2)*8..(c%2)*8+8. Each core computes its 8 heads' attention and a
partial out-projection [2048,1024]; host sums the two head-group partials per
batch and adds bout.

Device kernel v2 (per core):
  Projections in f32r (K=128 stationary tiles stream at ~216ns/512 rows).
  Q is stored zero-padded per head (other head's 64 rows = 0) so the S matmul
  keeps a K=128 stationary tile (K=64 stationaries reload ~2x slower).
  S^T[k,q] tiles f32r with causal q-trimming; exp on ACT -> bf16 SBUF ring;
  diagonal 128-col chunk masked via one shared triangular bf16 mask on DVE.
  ctx accumulated in [q, 65] layout: bf16 N=65 matmuls (expS chunk as lhsT,
  [V|1] as rhs); col 64 = softmax denominator; normalize via strided
  reciprocal + per-partition scalar multiply -> cn[q, feat] bf16.
  cn transposed on PE (bf16, via identity) -> cnT[feat, q]; out-projection
  bf16 with wout chunks as moving operand -> Y[tok, out] f32.
"""
import numpy as np

B, T, C = 4, 2048, 1024
H, HD = 16, 64
NCORES = 8


def _build_program():
    import concourse.bacc as bacc
    import concourse.tile as tile
    from concourse import mybir, masks as cmasks

    dtf = mybir.dt.float32
    dtr = mybir.dt.float32r
    dtb = mybir.dt.bfloat16
    EXP = mybir.ActivationFunctionType.Exp
    MULT = mybir.AluOpType.mult

    nc = bacc.Bacc('TRN2', target_bir_lowering=False, debug=False)
    xt_d = nc.dram_tensor("xt", [1024, 2048], dtr, kind="ExternalInput").ap()
    wqk_d = nc.dram_tensor("wqk", [1024, 1024], dtr, kind="ExternalInput").ap()
    wv_d = nc.dram_tensor("wv", [1024, 512], dtr, kind="ExternalInput").ap()
    wout_d = nc.dram_tensor("wout", [512, 1024], dtb, kind="ExternalInput").ap()
    mask_d = nc.dram_tensor("mask", [128, 128], dtb, kind="ExternalInput").ap()
    bqk_d = nc.dram_tensor("bqk", [1024], dtf, kind="ExternalInput").ap()
    bv_d = nc.dram_tensor("bv", [512], dtf, kind="ExternalInput").ap()
    y_d = nc.dram_tensor("y", [2048, 1024], dtf, kind="ExternalOutput").ap()

    with tile.TileContext(nc) as tc:
        with tc.tile_pool(name="const", bufs=1) as const, \
             tc.tile_pool(name="xt_p", bufs=8) as xt_p, \
             tc.tile_pool(name="ring", bufs=12) as ring_p, \
             tc.tile_pool(name="mx_p", bufs=4) as mx_p, \
             tc.tile_pool(name="cn_p", bufs=8) as cn_p, \
             tc.tile_pool(name="cnT_p", bufs=8) as cnT_p, \
             tc.tile_pool(name="r_p", bufs=3) as r_p, \
             tc.tile_pool(name="y_p", bufs=3) as y_p, \
             tc.tile_pool(name="ps_s", bufs=2, space="PSUM") as ps_s, \
             tc.tile_pool(name="ps_ctx", bufs=2, space="PSUM") as ps_ctx, \
             tc.tile_pool(name="ps_big", bufs=2, space="PSUM") as ps_big:

            # ---- constants / weights ----
            wqk_sb = const.tile([128, 8, 8, 128], dtr)   # [p, kc, j, c]
            wv_sb = const.tile([128, 8, 512], dtr)       # [p, kc, n]
            wout_sb = const.tile([128, 4, 2, 512], dtb)  # [p, fc, oc, c]
            maskb = const.tile([128, 128], dtb)
            bqk_sb = const.tile([128, 8], dtf)
            bv_sb = const.tile([128, 4], dtf)
            ident = const.tile([128, 128], dtb)
            vcol_f = const.tile([128, 16, 8, 1], dtf)
            kt_store = const.tile([128, 4, 4, 512], dtr)  # [p, j, tt, t]
            v_all = const.tile([128, 16, 8, 65], dtb)     # [p, kt, h, d|1]
            qs0 = const.tile([128, 4, 2, 512], dtr)       # [p, hp, hb, q]
            qs1 = const.tile([128, 4, 2, 512], dtr)
            qs = [qs0, qs1]

            # interleave x-tile-0 and j-major weight loads in consumption
            # order: chain (j=0, kc) needs xt[kc] + wqk j0 only
            xts0 = []
            xt_t0 = xt_p.tile([128, 512], dtr, tag="xt", name="xt_t0")
            nc.sync.dma_start(xt_t0[:], xt_d[0:128, 0:512])
            xts0.append(xt_t0)
            nc.sync.dma_start(wqk_sb[:, :, 0, :],
                              wqk_d[:, 0:128]
                              .rearrange("(kc p) c -> p kc c", p=128))
            for kc in range(1, 8):
                xt_t0 = xt_p.tile([128, 512], dtr, tag="xt", name="xt_t0")
                nc.sync.dma_start(xt_t0[:], xt_d[kc * 128:(kc + 1) * 128, 0:512])
                xts0.append(xt_t0)
            nc.sync.dma_start(bqk_sb[:], bqk_d.rearrange("(j p) -> p j", p=128))
            for j in range(1, 8):
                nc.sync.dma_start(wqk_sb[:, :, j, :],
                                  wqk_d[:, j * 128:(j + 1) * 128]
                                  .rearrange("(kc p) c -> p kc c", p=128))
            nc.sync.dma_start(wv_sb[:],
                              wv_d.rearrange("(kc p) n -> p kc n", p=128))
            nc.sync.dma_start(wout_sb[:],
                              wout_d.rearrange("(fc p) (oc c) -> p fc oc c",
                                               p=128, c=512))
            nc.sync.dma_start(maskb[:], mask_d)
            nc.sync.dma_start(bv_sb[:], bv_d.rearrange("(fc p) -> p fc", p=128))
            cmasks.make_identity(nc, ident[:])
            nc.vector.memset(vcol_f[:], 1.0)
            nc.vector.tensor_copy(v_all[:, :, :, 64:65], vcol_f[:])
            zt = const.tile([128, 512], dtf)
            nc.vector.memset(zt[:], 0.0)
            zb = const.tile([128, 128], dtb)
            nc.vector.memset(zb[:], 0.0)
            junk260 = const.tile([128, 260], dtb)
            nc.vector.memset(junk260[:], 1.0)
            for q_ in (qs0, qs1):
                for j_ in range(4):
                    for hb_ in range(2):
                        nc.vector.tensor_copy(q_[:, j_, hb_, :], zt[:])

            def dma_xt(tt, xts):
                for kc in range(8):
                    xt_t = xt_p.tile([128, 512], dtr, tag="xt", name="xt_t")
                    nc.sync.dma_start(
                        xt_t[:], xt_d[kc * 128:(kc + 1) * 128,
                                      tt * 512:(tt + 1) * 512])
                    xts.append(xt_t)

            def proj_q(tt, xts):
                par = tt % 2
                for j in range(4):
                    psq = ps_big.tile([128, 512], dtf, tag="big", name="psq")
                    for kc in range(8):
                        nc.tensor.matmul(psq[:], wqk_sb[:, kc, j, :],
                                         xts[kc][:], start=(kc == 0),
                                         stop=(kc == 7)).annotate('mm_q')
                    nc.vector.tensor_scalar_add(
                        qs[par][0:64, j, 0, :], psq[0:64, :],
                        bqk_sb[0:64, j:j + 1])
                    nc.vector.tensor_scalar_add(
                        qs[par][64:128, j, 1, :], psq[64:128, :],
                        bqk_sb[64:128, j:j + 1])

            def proj_kv(tt, xts):
                for j in range(4):
                    psk = ps_big.tile([128, 512], dtf, tag="big", name="psk")
                    for kc in range(8):
                        nc.tensor.matmul(psk[:], wqk_sb[:, kc, 4 + j, :],
                                         xts[kc][:], start=(kc == 0),
                                         stop=(kc == 7)).annotate('mm_k')
                    nc.vector.tensor_scalar_add(kt_store[:, j, tt, :], psk[:],
                                                bqk_sb[:, 4 + j:5 + j])
                for sub in range(4):
                    vt = tt * 4 + sub
                    psv = ps_big.tile([128, 512], dtf, tag="big", name="psv")
                    for kc in range(8):
                        nc.tensor.matmul(psv[:],
                                         xts[kc][:, sub * 128:(sub + 1) * 128],
                                         wv_sb[:, kc, :],
                                         start=(kc == 0),
                                         stop=(kc == 7)).annotate('mm_v')
                    nc.vector.tensor_copy(
                        v_all[:, vt, :, 0:64],
                        psv[:].rearrange("p (h d) -> p h d", h=8))

            def attention_hp(qt, hp, cn_tiles):
                par = qt % 2
                n_kt = 4 * qt + 4
                ctx_of = {}
                for hb in range(2):
                    cx = ps_ctx.tile([128, 260], dtf, tag="ctx", name="cx")
                    # zero the bank with one matmul, then accumulate with
                    # start=False only; interleaved per-region start/stop
                    # chains lose earlier chains' in-flight rounds (see mb8).
                    nc.tensor.matmul(cx[:, 0:260], zb[:], junk260[:],
                                     start=True, stop=False,
                                     skip_group_check=True).annotate('mm_z')
                    ctx_of[hb] = cx
                for ktg in range(n_kt):
                    ktt, kj = ktg // 4, ktg % 4
                    di = ktg - 4 * qt
                    if di < 0:
                        off = 0
                    else:
                        off = min(128 * di, 256)
                    nn = 512 - off
                    lhsT = kt_store[:, hp, ktt, kj * 128:(kj + 1) * 128]
                    # both heads' S in one 2-bank psum pair; one (or two)
                    # ACT exp instruction(s) covering both
                    s_ps = ps_s.tile([128, 1024], dtf, tag="s", name="s_ps")
                    e = ring_p.tile([128, 1024], dtb, tag="e", name="e")
                    for hb in range(2):
                        nc.tensor.matmul(
                            s_ps[:, 512 * hb:512 * hb + nn], lhsT,
                            qs[par][:, hp, hb, off:512],
                            start=True, stop=True).annotate('mm_s')
                    if di < 0:
                        rc_d, qc0 = 0, 0
                    else:
                        qc0 = di
                        rc_d = 128 * di - off
                    ne = nn - rc_d
                    if di < 2:
                        # single exp across both banks (incl. dead gap)
                        nc.scalar.activation(e[:, rc_d:512 + rc_d + ne],
                                             s_ps[:, rc_d:512 + rc_d + ne],
                                             EXP)
                    else:
                        nc.scalar.activation(e[:, rc_d:rc_d + ne],
                                             s_ps[:, rc_d:rc_d + ne], EXP)
                        nc.scalar.activation(e[:, 512 + rc_d:512 + rc_d + ne],
                                             s_ps[:, 512 + rc_d:512 + rc_d + ne],
                                             EXP)
                    mexps = {}
                    if di >= 0:
                        for hb in range(2):
                            mexp = mx_p.tile([128, 128], dtb, tag="mx",
                                             name="mexp")
                            nc.vector.tensor_tensor(
                                mexp[:], e[:, 512 * hb + rc_d:512 * hb + rc_d + 128],
                                maskb[:], MULT)
                            mexps[hb] = mexp
                    for hb in range(2):
                        head = 2 * hp + hb
                        for qc in range(qc0, 4):
                            rc = 128 * qc - off
                            lh = (mexps[hb][:] if (di >= 0 and qc == qc0)
                                  else e[:, 512 * hb + rc:512 * hb + rc + 128])
                            nc.tensor.matmul(
                                ctx_of[hb][:, qc * 65:qc * 65 + 65],
                                lh, v_all[:, ktg, head, :],
                                start=False, stop=False,
                                skip_group_check=True).annotate('mm_ctx')
                # normalize
                for hb in range(2):
                    head = 2 * hp + hb
                    rcp = r_p.tile([128, 4], dtf, tag="r", name="rcp")
                    nc.vector.reciprocal_approx_fast(
                        rcp[:], ctx_of[hb][:, 64:260:65])
                    for qc in range(4):
                        nc.vector.tensor_scalar_mul(
                            cn_tiles[qc][:, head * 64:head * 64 + 64],
                            ctx_of[hb][:, qc * 65:qc * 65 + 64],
                            rcp[:, qc:qc + 1])

            def transp_out_half(qt, cn_tiles, cnT_tiles, mis):
                if not cnT_tiles:
                    for fc in range(4):
                        tp = ps_big.tile([128, 512], dtb, tag="big", name="tp",
                                         padded_shape=[128, 512])
                        for qc in range(4):
                            nc.tensor.transpose(
                                tp[:, qc * 128:(qc + 1) * 128],
                                cn_tiles[qc][:, fc * 128:(fc + 1) * 128],
                                ident[:]).annotate('mm_t')
                        cnT = cnT_p.tile([128, 512], dtb, tag="ct", name="cnT")
                        nc.vector.tensor_scalar_add(cnT[:], tp[:],
                                                    bv_sb[:, fc:fc + 1])
                        cnT_tiles.append(cnT)
                for mi in mis:
                    for oc in range(2):
                        psy = ps_big.tile([128, 512], dtf, tag="big",
                                          name="psy", padded_shape=[128, 512])
                        for fc in range(4):
                            nc.tensor.matmul(
                                psy[:],
                                cnT_tiles[fc][:, mi * 128:(mi + 1) * 128],
                                wout_sb[:, fc, oc, :],
                                start=(fc == 0), stop=(fc == 3)).annotate('mm_y')
                        y_sb = y_p.tile([128, 512], dtf, tag="y", name="y_sb")
                        nc.vector.tensor_copy(y_sb[:], psy[:])
                        nc.sync.dma_start(
                            y_d[qt * 512 + mi * 128: qt * 512 + (mi + 1) * 128,
                                oc * 512:(oc + 1) * 512],
                            y_sb[:])

            # ---- main schedule ----
            proj_q(0, xts0)
            proj_kv(0, xts0)
            cns_prev = None
            cnT_prev = None
            for qt in range(4):
                cn_tiles = [cn_p.tile([128, 512], dtb, tag="cn", name="cn")
                            for _ in range(4)]
                xts_next = []
                if qt < 3:
                    dma_xt(qt + 1, xts_next)
                for hp in range(4):
                    attention_hp(qt, hp, cn_tiles)
                    if hp == 0 and qt < 3:
                        proj_q(qt + 1, xts_next)
                    elif hp == 1 and qt < 3:
                        proj_kv(qt + 1, xts_next)
                    if cns_prev is not None:
                        if hp == 2:
                            cnT_prev = []
                            transp_out_half(qt - 1, cns_prev, cnT_prev, (0, 1))
                        elif hp == 3:
                            transp_out_half(qt - 1, cns_prev, cnT_prev, (2, 3))
                cns_prev = cn_tiles
            cnT_last = []
            transp_out_half(3, cns_prev, cnT_last, (0, 1, 2, 3))
    nc.compile()
    return nc


def _host_shards(x, Wqkv, bqkv, Wout):
    import ml_dtypes
    kk = np.arange(128)[:, None]
    qq = np.arange(128)[None, :]
    mask = (qq >= kk).astype(ml_dtypes.bfloat16)

    in_maps = []
    for c in range(NCORES):
        b, hg = c // 2, c % 2
        s = hg * 512
        xt = np.ascontiguousarray(x[b].T)
        wqk = np.ascontiguousarray(
            np.concatenate([Wqkv[:, s:s + 512] * 0.125,
                            Wqkv[:, 1024 + s:1024 + s + 512]], axis=1))
        wv = np.ascontiguousarray(Wqkv[:, 2048 + s:2048 + s + 512])
        wout = np.ascontiguousarray(Wout[s:s + 512, :]).astype(ml_dtypes.bfloat16)
        bqk = np.concatenate([bqkv[s:s + 512] * 0.125,
                              bqkv[1024 + s:1024 + s + 512]]).astype(np.float32)
        bv = np.ascontiguousarray(bqkv[2048 + s:2048 + s + 512]).astype(np.float32)
        in_maps.append({"xt": xt, "wqk": wqk, "wv": wv, "wout": wout,
                       "mask": mask, "bqk": bqk, "bv": bv})
    return in_maps


_CACHED = {}


def kernel(x, Wqkv, bqkv, Wout, bout):
    from concourse.bass_utils import run_bass_kernel_spmd

    x = np.asarray(x, dtype=np.float32)
    Wqkv = np.asarray(Wqkv, dtype=np.float32)
    bqkv = np.asarray(bqkv, dtype=np.float32)
    Wout = np.asarray(Wout, dtype=np.float32)
    bout = np.asarray(bout, dtype=np.float32)
    assert x.shape == (B, T, C), x.shape

    if 'nc' not in _CACHED:
        _CACHED['nc'] = _build_program()
    nc = _CACHED['nc']

    in_maps = _host_shards(x, Wqkv, bqkv, Wout)
    res = run_bass_kernel_spmd(nc, in_maps, core_ids=list(range(NCORES)))

    y = np.empty((B, T, C), np.float32)
    for b in range(B):
        y[b] = res.results[2 * b]["y"] + res.results[2 * b + 1]["y"] + bout
    return y
